# revision 18
# baseline (speedup 1.0000x reference)
"""Trainium2 Bass kernel for nn_BartAttention_66786741453241 (8 NeuronCores).

Reference (bugs preserved): no softmax — raw attention scores are used for the
AV matmul, and q is scaled by dh**-0.5 with scores further divided by sqrt(dh),
net 1/dh. The whole computation is therefore LINEAR in V, so we reassociate
    (Q K^T / 64) V  ==  Q (K^T V) / 64
which collapses the [T,T] score matrices into per-head [64,64] K^T V matrices
(~32x fewer attention FLOPs, exact in infinite precision).

Sharding: tensor-parallel by (batch, head-group) — core i handles batch i//4
and heads 4*(i%4) .. 4*(i%4)+4 for ALL 2048 tokens of that batch:
  - fused k|v projection (concatenated weight slice) -> per-head K^T V is
    complete locally: NO collective anywhere,
  - block-diagonal pair tiles of V^T K feed M_j = blockdiag(KTV) @ WoT_pair,
    so the tail is one matmul family: partial out^T = sum_j M_j^T @ qT_j,
  - qT projection for its 4 heads (bias + the net 1/64 scaling folded in),
  - partial out^T (bf16) DMA'd out per core.
The host sums the 4 partials per batch and adds bo — that host-side reduce is
the unshard step for the out_proj input-dim sharding (the "all-reduce after
out_proj" of the standard tensor-parallel recipe).
Other details: PE warm-up via dummy matmuls during the input-DMA wait (keeps
the HAM clock-gate at 2.4 GHz for the real stream), inputs in consumption-
order quarter tiles, all matmuls bf16 (fp32 PSUM accumulate); end-to-end
relative error vs the f32 reference ~4.7e-3 (gate 2e-2).
"""

import os
import sys
import types

import numpy as np
import ml_dtypes

import concourse.bacc as bacc
import concourse.mybir as mybir
import concourse.tile as tile
from concourse.bass_utils import run_bass_kernel_spmd

BF16 = mybir.dt.bfloat16
F32 = mybir.dt.float32
NPBF16 = ml_dtypes.bfloat16

E = 1024        # embed dim
H = 16          # heads
DH = 64         # head dim
B, T = 2, 2048
NC = 8          # cores
P = 128
KC = E // P     # 8 contraction chunks for the in-projections
HPC = 4         # heads per core
EH = HPC * DH   # 256: per-core q/k/v feature width
TG = T // 512   # 4 moving-dim groups of 512 tokens
TTC = T // P    # 16 token chunks per core
Ident = mybir.ActivationFunctionType.Identity
OUT_BF16 = True  # partial out^T in bf16 (halves the output DMA)


def _install_axon_profile_hook():
    """Make trace=True usable under axon: register the NTFF hook that the
    staged antenv lacks, and neuter artifact upload (no bucket here). Safe
    no-op when pieces are missing."""
    try:
        import concourse.bass_utils as bu
        bu.upload_artifacts = lambda tmpdir: "local://" + tmpdir
    except Exception:
        pass
    if "antenv.axon_hooks" in sys.modules:
        return
    hook = None
    try:
        from trn_agent_boot.trn_boot import _ntff_profile_via_ctypes
        so = "/opt/axon/libaxon_pjrt.so"
        if os.path.exists(so):
            hook = _ntff_profile_via_ctypes(so)
    except Exception:
        hook = None
    mod = types.ModuleType("antenv.axon_hooks")
    mod.get_axon_ntff_profile_hook = lambda: hook
    mod.set_axon_ntff_profile_hook = lambda h: None
    sys.modules["antenv.axon_hooks"] = mod


def build():
    """Build + compile the per-core SPMD graph (identical on all 8 cores)."""
    nc = bacc.Bacc("TRN2", target_bir_lowering=False, debug=False, num_devices=NC)

    out_dt = BF16 if OUT_BF16 else F32
    hsT = nc.dram_tensor("hsT", [E, T], BF16, kind="ExternalInput")       # 4 MB
    wkvt = nc.dram_tensor("wkvt", [E, 2 * EH], BF16, kind="ExternalInput")  # 1 MB
    wqt = nc.dram_tensor("wqt", [E, EH], BF16, kind="ExternalInput")      # 0.5 MB
    wot = nc.dram_tensor("wot", [EH, E], BF16, kind="ExternalInput")      # 0.5 MB
    # pre-tiled biases: cols 0..512 = k|v bias rows, cols 512..514 = bq/64
    bkvb = nc.dram_tensor("bkvb", [P, 2 * EH + 16], F32, kind="ExternalInput")
    outT = nc.dram_tensor("outT", [E, T], out_dt, kind="ExternalOutput")

    with tile.TileContext(nc) as tc:
        with (
            tc.tile_pool(name="sb", bufs=1) as sb,
            tc.tile_pool(name="stg", bufs=3) as stg,
            tc.tile_pool(name="psA", bufs=7, space="PSUM") as psA,
            tc.tile_pool(name="psB", bufs=1, space="PSUM") as psB,
        ):
            # ---- PE warm-up: dummy matmuls on memset tiles keep the PE's
            # HAM activity window busy during the input-DMA wait (the HAM
            # clock gate only grants 2.4 GHz after ~3.4us of sustained PE
            # activity; idle >3.4us re-throttles to 1.2 GHz).
            dum_w = sb.tile([P, P], BF16, tag="dum_w")
            nc.gpsimd.memset(dum_w[:], 0.0)
            dum_x = sb.tile([P, 512], BF16, tag="dum_x")
            nc.gpsimd.memset(dum_x[:], 0.0)
            dum_ps = psB.tile([P, 512], F32, tag="psB")
            # 5 N=512 dummies bridge PE activity from body-start (~7.8us)
            # to first-chunk data-ready (~10us). The real KV stream then
            # starts COLD (K=4/8): half-pace consumption lets the DMA
            # supply build a buffer for free while the cold cycles overlap
            # time we would have spent waiting anyway; the HAM warm grant
            # (~3.4us after activity onset) lands ~11.2us into continuous
            # dummy->real activity.
            for _ in range(5):
                nc.tensor.matmul(dum_ps[:], dum_w[:], dum_x[:], start=True, stop=True)
            dum_out = sb.tile([P, 4], BF16, tag="dum_out")
            nc.vector.tensor_copy(dum_out[:], dum_ps[:, 0:4])

            # ---- loads. hs/wkv stream on the Sync queue, strictly in KV
            # consumption order at per-chunk granularity (first matmul gated
            # on just 0.375 MB; no group-arrival stalls). The weight/bias
            # side (bkvb, wq, wo) triggers from the otherwise-idle Scalar
            # queue: its expensive multi-chunk descriptor generation (the
            # rearranged wq DMA costs ~3.5 us) must never delay an hs
            # trigger, and its transfers ride a separate DMA queue.
            wkv_t = [
                sb.tile([P, 2 * EH], BF16, tag=f"wkv{c}", name=f"wkv{c}")
                for c in range(KC)
            ]
            hs_t = [
                sb.tile([P, T], BF16, tag=f"hs{c}", name=f"hs{c}") for c in range(KC)
            ]
            bkv_sb = sb.tile([P, 2 * EH + 16], F32, tag="bkv")
            wq_big = sb.tile([P, KC * EH], BF16, tag="wq")
            wo_sb = [sb.tile([P, E], BF16, tag=f"wo{c}", name=f"wo{c}") for c in range(2)]

            wkv3 = wkvt.ap().rearrange("(c p) n -> p c n", p=P)
            hs3 = hsT.ap().rearrange("(c p) t -> p c t", p=P)

            nc.scalar.dma_start(bkv_sb[:], bkvb[:, :])

            for c in range(KC):
                nc.sync.dma_start(wkv_t[c][:], wkv3[:, c, :])
                if c == 0:
                    nc.sync.dma_start(hs_t[0][:, 0:1024], hs3[:, 0, 0:1024])
                    nc.sync.dma_start(hs_t[0][:, 1024:T], hs3[:, 0, 1024:T])
                else:
                    nc.sync.dma_start(hs_t[c][:], hs3[:, c, :])
            # wq/wo ride the same FIFO queue BEHIND the full hs stream: their
            # 1 MB transfers start only after the last hs chunk (~22us), so
            # they cannot steal supply bandwidth from the rate-critical KV
            # stream, yet still arrive long before the Q/M phases need them.
            nc.sync.dma_start(
                wq_big[:].rearrange("p (c n) -> p c n", c=KC),
                wqt.ap().rearrange("(c p) n -> p c n", p=P),
            )
            for c in range(2):
                nc.sync.dma_start(wo_sb[c][:], wot[c * P:(c + 1) * P, :])

            def hs_c(c):
                return hs_t[c][:]

            def wkv_c(c):
                return wkv_t[c][:]

            # ---- fused k|v projection: [128 tokens, k(4 heads)|v(4 heads)]
            kv_sb = [
                sb.tile([P, 2 * EH], BF16, tag=f"kv{tt}", name=f"kv{tt}")
                for tt in range(TTC)
            ]
            for tt in range(TTC):
                ps = psA.tile([P, 512], F32, tag="psA")
                for c in range(KC):
                    nc.tensor.matmul(
                        ps[:],
                        hs_c(c)[:, tt * P:(tt + 1) * P],
                        wkv_c(c),
                        start=(c == 0),
                        stop=(c == KC - 1),
                    )
                nc.vector.tensor_add(kv_sb[tt][:], ps[:], bkv_sb[:, 0:2 * EH])

            # ---- per-head K^T V (full batch, local: no collective)
            # head pairs stacked on partitions: head 2j+hh at rows hh*64,
            # cols j*64 — bases line up with qT slices in the Q@KTV matmul.
            # pair-packed V^T K: ONE MM per (pair, chunk) — lhsT = [v_A|v_B]
            # (M=128) against rhs = [k_A|k_B] (N=128); the useful diagonal
            # [64,64] blocks (VTK_h = KTV_h^T) land in a zeroed block-diagonal
            # [128,128] tile per pair.
            vtk_bd = [
                sb.tile([P, P], BF16, tag=f"vtk_bd{j}", name=f"vtk_bd{j}")
                for j in range(HPC // 2)
            ]
            for j in range(HPC // 2):
                nc.gpsimd.memset(vtk_bd[j][:], 0.0)
            for j in range(HPC // 2):
                ps = psB.tile([P, 2 * DH], F32, tag="psB")
                for tt in range(TTC):
                    nc.tensor.matmul(
                        ps[:],
                        kv_sb[tt][:, EH + 2 * j * DH:EH + (2 * j + 2) * DH],
                        kv_sb[tt][:, 2 * j * DH:(2 * j + 2) * DH],
                        start=(tt == 0),
                        stop=(tt == TTC - 1),
                    )
                nc.vector.tensor_copy(vtk_bd[j][0:DH, 0:DH], ps[0:DH, 0:DH])
                nc.vector.tensor_copy(
                    vtk_bd[j][DH:2 * DH, DH:2 * DH], ps[DH:2 * DH, DH:2 * DH]
                )
            # fold the out-projection through KTV once per pair:
            # M_j = blockdiag(KTV_A, KTV_B) @ WoT_pair  (out^T = sum_j M_j^T qT_j)
            m_sb = [
                sb.tile([P, E], BF16, tag=f"m{j}", name=f"m{j}")
                for j in range(HPC // 2)
            ]
            for j in range(HPC // 2):
                for half in range(2):
                    ps = psB.tile([P, 512], F32, tag="psB")
                    nc.tensor.matmul(
                        ps[:],
                        vtk_bd[j][:],
                        wo_sb[j][:, half * 512:(half + 1) * 512],
                        start=True,
                        stop=True,
                    )
                    nc.vector.tensor_copy(
                        m_sb[j][:, half * 512:(half + 1) * 512], ps[:]
                    )

            # ---- qT projection [e_out 256, tokens], bias + 1/64 folded
            q_sb = [
                sb.tile([P, T], BF16, tag=f"q{m}", name=f"q{m}")
                for m in range(EH // P)
            ]
            for m in range(EH // P):
                for tg in range(TG):
                    ps = psA.tile([P, 512], F32, tag="psA")
                    for c in range(KC):
                        nc.tensor.matmul(
                            ps[:],
                            wq_big[:, c * EH + m * P:c * EH + (m + 1) * P],
                            hs_c(c)[:, tg * 512:(tg + 1) * 512],
                            start=(c == 0),
                            stop=(c == KC - 1),
                        )
                    nc.scalar.activation(
                        q_sb[m][:, tg * 512:(tg + 1) * 512], ps[:], Ident,
                        bias=bkv_sb[:, 2 * EH + m:2 * EH + m + 1], scale=1.0 / 64.0,
                    )

            # ---- partial out^T = sum_j M_j^T @ qT_j (no bias: host adds bo)
            # Output DMA triggers go on the near-idle GpSimd queue so they
            # never serialize behind input triggers on Sync. The last chunk
            # DMAs per 512-token group, and its final group's PSUM
            # evacuation is split across Vector+Scalar so the copy->DMA
            # tail after the last matmul is minimal.
            for m in range(KC):
                o_stage = stg.tile([P, T], out_dt, tag="ostg")
                last = m == KC - 1
                for tg in range(TG):
                    ps = psA.tile([P, 512], F32, tag="psA")
                    for c in range(2):
                        nc.tensor.matmul(
                            ps[:],
                            m_sb[c][:, m * P:(m + 1) * P],
                            q_sb[c][:, tg * 512:(tg + 1) * 512],
                            start=(c == 0),
                            stop=(c == 1),
                        )
                    sl = slice(tg * 512, (tg + 1) * 512)
                    if last and tg == TG - 1:
                        # split the very last evacuation across both engines
                        # so the post-matmul tail is copy/2 + trigger + a
                        # 64 KB transfer.
                        lo = slice(tg * 512, tg * 512 + 256)
                        hi = slice(tg * 512 + 256, (tg + 1) * 512)
                        nc.vector.tensor_copy(o_stage[:, lo], ps[:, 0:256])
                        nc.scalar.copy(o_stage[:, hi], ps[:, 256:512])
                        nc.sync.dma_start(outT[m * P:(m + 1) * P, lo], o_stage[:, lo])
                        nc.sync.dma_start(outT[m * P:(m + 1) * P, hi], o_stage[:, hi])
                    else:
                        if tg % 2 == 0:
                            nc.vector.tensor_copy(o_stage[:, sl], ps[:])
                        else:
                            nc.scalar.copy(o_stage[:, sl], ps[:])
                        if last and tg == 1:
                            # first half of the last chunk in one early DMA
                            nc.sync.dma_start(
                                outT[m * P:(m + 1) * P, 0:1024], o_stage[:, 0:1024]
                            )
                        elif last and tg == 2:
                            nc.sync.dma_start(
                                outT[m * P:(m + 1) * P, sl], o_stage[:, sl]
                            )
                # The last chunk's DMAs all trigger from Sync (idle once
                # inputs are done, and its end-of-kernel DRAIN is cheap);
                # GpSimd carries only the earlier chunks, so its expensive
                # multi-us DRAIN poll finishes before the last matmul.
                if not last:
                    nc.gpsimd.dma_start(outT[m * P:(m + 1) * P, :], o_stage[:])

    nc.compile()
    return nc


_NC_CACHE = None


def _get_nc():
    global _NC_CACHE
    if _NC_CACHE is None:
        _install_axon_profile_hook()
        _NC_CACHE = build()
    return _NC_CACHE


def bias_tile(bkv, bq_slice):
    """[512] kv-bias + [256] scaled q-bias -> one [128, 528] f32 DMA tile:
    cols 0..512 = kv bias broadcast rows, cols 512..514 = bq/64 chunks."""
    t = np.zeros((P, 2 * EH + 16), np.float32)
    t[:, 0:2 * EH] = bkv
    t[:, 2 * EH:2 * EH + 2] = bq_slice.reshape(2, P).T
    return t


def make_in_maps(hidden_states, Wq, bq, Wk, bk, Wv, bv, Wo, bo):
    f32 = np.float32
    hs = np.asarray(hidden_states, f32)
    WqT = np.asarray(Wq, f32).T    # [e_in, e_out]
    WkT = np.asarray(Wk, f32).T
    WvT = np.asarray(Wv, f32).T
    WoT = np.asarray(Wo, f32).T
    bq64 = np.asarray(bq, f32) / 64.0
    bk = np.asarray(bk, f32)
    bv = np.asarray(bv, f32)

    hsT_b = [
        np.ascontiguousarray(hs[b].T).astype(NPBF16) for b in range(B)
    ]
    in_maps = []
    for i in range(NC):
        g, r = divmod(i, HPC)
        sl = slice(r * EH, (r + 1) * EH)
        wkvt = np.concatenate([WkT[:, sl], WvT[:, sl]], axis=1)
        bkv = np.concatenate([bk[sl], bv[sl]])
        in_maps.append({
            "hsT": hsT_b[g],
            "wkvt": np.ascontiguousarray(wkvt).astype(NPBF16),
            "wqt": np.ascontiguousarray(WqT[:, sl]).astype(NPBF16),
            "wot": np.ascontiguousarray(WoT[sl, :]).astype(NPBF16),
            "bkvb": bias_tile(bkv, bq64[sl]),
        })
    return in_maps


def run(inputs, trace=False, **kw):
    """Run on 8 NeuronCores; returns (full_output [B,T,E] f32, BassKernelResults)."""
    nc = _get_nc()
    in_maps = make_in_maps(**inputs)
    try:
        res = run_bass_kernel_spmd(nc, in_maps, list(range(NC)), trace=trace, **kw)
    except Exception:
        # rare transient NRT_EXEC_UNIT_UNRECOVERABLE — one retry usually lands
        res = run_bass_kernel_spmd(nc, in_maps, list(range(NC)), trace=trace, **kw)
    bo = np.asarray(inputs["bo"], np.float32)
    out = np.empty((B, T, E), np.float32)
    for g in range(B):
        acc = res.results[g * HPC]["outT"].astype(np.float32)
        for r in range(1, HPC):
            acc = acc + res.results[g * HPC + r]["outT"].astype(np.float32)
        out[g] = acc.T + bo
    return out, res


def kernel(**inputs):
    out, _ = run(inputs, trace=False)
    return out

